# revision 1
# baseline (speedup 1.0000x reference)
"""Trainium2 Bass kernel for a dense transformer decoder layer.

Tensor-parallel over 8 NeuronCores: each core owns 4 q-heads, 1 kv-head and
a 1/8 slice of the FFN hidden dim. One on-device AllReduce after the
attention output projection (with x/8 folded in so the AR result is the
residual h2 directly); the final FFN partial sums are combined on the host.

Layout convention: activations are kept transposed as [feature, token] so the
contraction dim of every matmul is already on SBUF partitions. q/k head dims
are de-interleaved (x0 block then x1 block) so RoPE acts on 32-row blocks.
All matmul operands are float32r (TF32-like, full PE rate at N>=256).
"""
import ml_dtypes
import numpy as np

import concourse.bass as bass
import concourse.bacc as bacc
import concourse.tile as tile
from concourse import mybir
from concourse.masks import make_identity
from concourse.tile_rust import add_dep_helper

F32 = mybir.dt.float32
F32R = mybir.dt.float32r
BF16 = mybir.dt.bfloat16
AF = mybir.ActivationFunctionType
OP = mybir.AluOpType

N_CORES = 8
EPS = 1e-5


def _cfg(S=2048, F=8192):
    B, E, HD = 2, 2048, 64
    T = B * S
    c = dict(B=B, S=S, E=E, F=F, HD=HD, T=T)
    c["KT_E"] = E // 128                 # k-tiles over E
    c["TCH"] = min(512, S)               # token chunk (== attention q chunk)
    c["NCH"] = T // c["TCH"]
    c["QC"] = min(512, S)                # attention q chunk
    c["NQC"] = S // c["QC"]
    c["KT_S"] = S // 128                 # k-tiles per batch (attention)
    c["Fc"] = F // N_CORES               # FFN rows per core
    c["FH"] = 2                          # FFN f-half phases
    c["FHR"] = c["Fc"] // 2              # rows per half
    c["FHM"] = c["FHR"] // 128           # m-tiles / k-tiles per half
    assert c["FHM"] >= 1
    return c


def build(cfg, collective=True):
    c = cfg
    E, T, TCH, NCH = c["E"], c["T"], c["TCH"], c["NCH"]
    KT_E, QC, NQC, KT_S = c["KT_E"], c["QC"], c["NQC"], c["KT_S"]
    B, S = c["B"], c["S"]
    Fc = c["Fc"]
    FM = Fc // 128                       # FFN hidden k/m tiles per core
    QKT = QC // 128                      # k-tiles inside one diagonal q chunk
    KT_C = TCH // 128                    # k-tiles per token chunk (attention V)

    nc = bacc.Bacc(None, target_bir_lowering=False, debug=False)

    # ---- I/O ----
    xT = nc.dram_tensor("xT", [E, T], F32R, kind="ExternalInput")
    xTb = nc.dram_tensor("xTb", [E, T], BF16, kind="ExternalInput")
    wqkvT = nc.dram_tensor("wqkvT", [E, 448], BF16, kind="ExternalInput")
    woT = nc.dram_tensor("woT", [256, E], F32R, kind="ExternalInput")
    w1T = nc.dram_tensor("w1T", [E, Fc], BF16, kind="ExternalInput")
    w3T = nc.dram_tensor("w3T", [E, Fc], BF16, kind="ExternalInput")
    w2T = nc.dram_tensor("w2T", [Fc, E], BF16, kind="ExternalInput")
    cosq = nc.dram_tensor("cosq", [128, T], F32, kind="ExternalInput")
    sinq = nc.dram_tensor("sinq", [128, T], F32, kind="ExternalInput")
    outT = nc.dram_tensor("outT", [E, T], F32, kind="ExternalOutput")

    replica_groups = [list(range(N_CORES))]

    with tile.TileContext(nc) as tc:
        with (
            tc.tile_pool(name="dram", bufs=1, space="DRAM") as dram,
        ):
            o_bounce = dram.tile([NCH, E, TCH], F32)
            h2_shl = []
            for _tch in range(NCH):
                h2c_t = dram.tile([E, TCH], F32, addr_space="Shared",
                                  tag=f"h2sh{_tch}")
                h2_shl.append(h2c_t)

            gps_cm = tc.tile_pool(name="gps", bufs=2, space="PSUM")
            gps = gps_cm.__enter__()
            # manually-scoped pools with nested lifetimes: ao > qk
            ao_cm = tc.tile_pool(name="ao", bufs=1)          # .. oproj end
            ao_pool = ao_cm.__enter__()
            qk_cm = tc.tile_pool(name="qk", bufs=1)          # .. attention end
            qk = qk_cm.__enter__()

            # q/k tiles: written by qkv matmul epilogue, roped in place.
            qr0 = qk.tile([128, T], F32R, tag="qr0")   # q heads 0,1
            qr1 = qk.tile([128, T], F32R, tag="qr1")   # q heads 2,3
            kr = qk.tile([128, T], F32R, tag="kr")     # kv head x2
            vaug = qk.tile([128, B * KT_S, 65], BF16, tag="vaug")

            # ---------- phase 1: qkv projection + rope + V transpose ----------
            with (
                tc.tile_pool(name="qkvw", bufs=1) as qkvw,
                tc.tile_pool(name="qkvx", bufs=2) as qkvx,
                tc.tile_pool(name="qkvs", bufs=2) as qkvs,
                tc.tile_pool(name="ropep", bufs=2) as ropep,
            ):
                wq_sb = qkvw.tile([128, KT_E, 448], BF16, tag="wq")
                for kt in range(KT_E):
                    nc.sync.dma_start(out=wq_sb[:, kt, :],
                                      in_=wqkvT[kt * 128:(kt + 1) * 128, :])
                ident_f = qkvw.tile([64, 64], F32, tag="ident_f")
                make_identity(nc, ident_f[:])
                ident = qkvw.tile([64, 64], F32R, tag="ident")
                nc.vector.tensor_copy(out=ident[:], in_=ident_f[:])
                ones_f = qkvw.tile([128, 1], F32, tag="ones_f")
                nc.vector.memset(ones_f[:], 1.0)
                ones_r = qkvw.tile([128, 1], F32R, tag="ones_r")
                nc.vector.tensor_copy(out=ones_r[:], in_=ones_f[:])
                eps1q = qkvw.tile([1, 1], F32, tag="eps1q")
                nc.vector.memset(eps1q[:], EPS)

                for tch in range(NCH):
                    t0 = tch * TCH
                    tsl = slice(t0, t0 + TCH)
                    xch = qkvx.tile([128, KT_E, TCH], BF16, tag="xch")
                    for kt in range(KT_E):
                        nc.sync.dma_start(
                            out=xch[:, kt, :],
                            in_=xTb[kt * 128:(kt + 1) * 128, tsl])
                    cos_sb = qkvs.tile([128, TCH], F32, tag="cos")
                    sin_sb = qkvs.tile([128, TCH], F32, tag="sin")
                    nc.sync.dma_start(out=cos_sb[:], in_=cosq[:, tsl])
                    nc.sync.dma_start(out=sin_sb[:], in_=sinq[:, tsl])
                    # norm1 scales for this chunk (sum of squares over E via PE)
                    ps1c = gps.tile([1, TCH], F32, tag="n1")
                    for kt in range(KT_E):
                        sqx = qkvs.tile([128, TCH], F32R, tag="sqx")
                        nc.scalar.activation(out=sqx[:], in_=xch[:, kt, :],
                                             func=AF.Square)
                        nc.tensor.matmul(ps1c[:], ones_r[:], sqx[:],
                                         start=(kt == 0), stop=(kt == KT_E - 1))
                    st1 = qkvs.tile([1, TCH], F32, tag="st1")
                    nc.scalar.activation(out=st1[:], in_=ps1c[:], func=AF.Sqrt,
                                         scale=1.0 / E, bias=eps1q[:])
                    r01 = qkvs.tile([1, TCH], F32, tag="r01")
                    nc.vector.reciprocal(out=r01[:], in_=st1[:])
                    t11 = qkvs.tile([1, TCH], F32, tag="t11")
                    nc.vector.tensor_tensor(out=t11[:], in0=st1[:], in1=r01[:],
                                            op=OP.mult)
                    nc.vector.tensor_scalar(out=t11[:], in0=t11[:], scalar1=-1.0,
                                            scalar2=2.0, op0=OP.mult, op1=OP.add)
                    rr1 = qkvs.tile([1, TCH], F32, tag="rr1")
                    nc.vector.tensor_tensor(out=rr1[:], in0=r01[:], in1=t11[:],
                                            op=OP.mult)
                    s1b = qkvs.tile([128, TCH], F32, tag="s1b")
                    nc.gpsimd.partition_broadcast(s1b[:], rr1[:])
                    vT_c = qkvs.tile([64, TCH], F32R, tag="vT_c")
                    for m, (dst, rows) in enumerate(
                            [(qr0, 128), (qr1, 128), (kr, 128), (vT_c, 64)]):
                        ps = gps.tile([128, TCH], F32, tag="mm")
                        for kt in range(KT_E):
                            nc.tensor.matmul(
                                ps[:rows, :],
                                wq_sb[:, kt, m * 128:m * 128 + rows],
                                xch[:, kt, :],
                                start=(kt == 0), stop=(kt == KT_E - 1))
                        if m < 3:
                            nc.vector.tensor_tensor(
                                out=dst[:rows, tsl], in0=ps[:rows, :],
                                in1=s1b[:rows, :], op=OP.mult)
                        else:
                            nc.vector.tensor_tensor(
                                out=vT_c[:], in0=ps[:rows, :],
                                in1=s1b[:rows, :], op=OP.mult)
                    # rope on this chunk (in place)
                    for qt in (qr0, qr1, kr):
                        swp = ropep.tile([128, TCH], F32, tag="swp")
                        for b0 in (0, 64):
                            nc.sync.dma_start(
                                out=swp[b0:b0 + 32, :],
                                in_=qt[b0 + 32:b0 + 64, tsl].bitcast(F32))
                            nc.sync.dma_start(
                                out=swp[b0 + 32:b0 + 64, :],
                                in_=qt[b0:b0 + 32, tsl].bitcast(F32))
                        tm = ropep.tile([128, TCH], F32, tag="tm")
                        nc.vector.tensor_tensor(out=tm[:], in0=qt[:, tsl].bitcast(F32),
                                                in1=cos_sb[:], op=OP.mult)
                        um = ropep.tile([128, TCH], F32, tag="um")
                        nc.vector.tensor_tensor(out=um[:], in0=swp[:],
                                                in1=sin_sb[:], op=OP.mult)
                        nc.vector.tensor_tensor(out=qt[:, tsl], in0=tm[:], in1=um[:],
                                                op=OP.add)
                    # V transpose for this chunk -> vaug (col 64 = ones)
                    for j in range(KT_C):
                        kt = tch * KT_C + j
                        pt = gps.tile([128, 64], F32R, tag="attv")
                        nc.tensor.transpose(pt[:], vT_c[:, j * 128:(j + 1) * 128],
                                            ident[:])
                        nc.vector.tensor_copy(out=vaug[:, kt, 0:64], in_=pt[:])
                        nc.vector.tensor_copy(out=vaug[:, kt, 64:65], in_=ones_f[:])

            # ---------- phase 2: attention -> o-proj -> chunked AR ----------
            aoT0 = ao_pool.tile([128, T], F32R, tag="aoT0")
            aoT1 = ao_pool.tile([128, T], F32R, tag="aoT1")
            with (
                tc.tile_pool(name="att", bufs=1) as att,
                tc.tile_pool(name="atts", bufs=2) as atts,
                tc.tile_pool(name="attw", bufs=3) as attw,
                tc.tile_pool(name="opo", bufs=2) as opo,
            ):
                for b in range(B):
                    for qc in range(NQC):
                        qs = b * S + qc * QC
                        n_kb = qc * QKT + QKT
                        for (qtile, aoT) in [(qr0, aoT0), (qr1, aoT1)]:
                            expsA = att.tile([128, KT_S, QC], BF16, tag="expsA")
                            expsB = att.tile([128, KT_S, QC], BF16, tag="expsB")
                            exps = [expsA, expsB]
                            for kb in range(n_kb):
                                ksl = slice(b * S + kb * 128, b * S + kb * 128 + 128)
                                for h in range(2):
                                    ps = gps.tile([128, QC], F32, tag="sc")
                                    nc.tensor.matmul(
                                        ps[:],
                                        kr[h * 64:(h + 1) * 64, ksl],
                                        qtile[h * 64:(h + 1) * 64, qs:qs + QC],
                                        start=True, stop=True)
                                    nc.scalar.activation(
                                        out=exps[h][:, kb, :], in_=ps[:], func=AF.Exp)
                                    j = kb - qc * QKT
                                    if j >= 0:
                                        nc.gpsimd.affine_select(
                                            out=exps[h][:, kb, :],
                                            in_=exps[h][:, kb, :],
                                            compare_op=OP.is_ge,
                                            fill=0.0, base=-128 * j,
                                            pattern=[[1, QC]], channel_multiplier=-1)
                            for h in range(2):
                                po = gps.tile([65, QC], F32, tag="attv")
                                for kb in range(n_kb):
                                    gkt = b * KT_S + kb
                                    nc.tensor.matmul(
                                        po[:], vaug[:, gkt, :], exps[h][:, kb, :],
                                        start=(kb == 0), stop=(kb == n_kb - 1))
                                # softmax denominators live in row 64
                                ssb = atts.tile([1, QC], F32, tag="ssb")
                                nc.vector.tensor_copy(out=ssb[:], in_=po[64:65, :])
                                r0 = atts.tile([1, QC], F32, tag="r0")
                                nc.vector.reciprocal(out=r0[:], in_=ssb[:])
                                t1 = atts.tile([1, QC], F32, tag="t1")
                                nc.vector.tensor_tensor(out=t1[:], in0=ssb[:],
                                                        in1=r0[:], op=OP.mult)
                                nc.vector.tensor_scalar(
                                    out=t1[:], in0=t1[:], scalar1=-1.0, scalar2=2.0,
                                    op0=OP.mult, op1=OP.add)
                                rr = atts.tile([1, QC], F32, tag="rr")
                                nc.vector.tensor_tensor(out=rr[:], in0=r0[:],
                                                        in1=t1[:], op=OP.mult)
                                rb = atts.tile([64, QC], F32, tag="rb")
                                nc.gpsimd.partition_broadcast(rb[:], rr[:])
                                nc.vector.tensor_tensor(
                                    out=aoT[h * 64:(h + 1) * 64, qs:qs + QC],
                                    in0=po[0:64, :], in1=rb[:], op=OP.mult)
                        # ---- o-proj + x/8 for this token chunk, then AR ----
                        tch = b * NQC + qc
                        t0 = tch * TCH
                        for em in range(KT_E):
                            wo_em = attw.tile([128, 2, 128], F32R, tag="wo_em")
                            for kt in range(2):
                                nc.sync.dma_start(
                                    out=wo_em[:, kt, :],
                                    in_=woT[kt * 128:(kt + 1) * 128,
                                            em * 128:(em + 1) * 128])
                            ps = gps.tile([128, TCH], F32, tag="mm")
                            for kt, ao_t in ((0, aoT0), (1, aoT1)):
                                nc.tensor.matmul(
                                    ps[:], wo_em[:, kt, :],
                                    ao_t[:, t0:t0 + TCH],
                                    start=(kt == 0), stop=(kt == 1))
                            x_em = opo.tile([128, TCH], F32, tag="x_em")
                            nc.sync.dma_start(
                                out=x_em[:],
                                in_=xT[em * 128:(em + 1) * 128,
                                       t0:t0 + TCH].bitcast(F32))
                            ob = opo.tile([128, TCH], F32, tag="ob")
                            nc.vector.scalar_tensor_tensor(
                                out=ob[:], in0=x_em[:], scalar=1.0 / N_CORES,
                                in1=ps[:], op0=OP.mult, op1=OP.add)
                            nc.sync.dma_start(
                                out=o_bounce[tch, em * 128:(em + 1) * 128, :],
                                in_=ob[:])
                        if collective:
                            nc.gpsimd.collective_compute(
                                "AllReduce", OP.add, replica_groups=replica_groups,
                                ins=[o_bounce[tch].opt()], outs=[h2_shl[tch].opt()])
                        else:
                            nc.sync.dma_start(out=h2_shl[tch][:], in_=o_bounce[tch])
            qk_cm.__exit__(None, None, None)
            ao_cm.__exit__(None, None, None)

            # ---------- phase 3: norm2 + FFN (fused, single pass) ----------
            with (
                tc.tile_pool(name="ffc", bufs=1) as ffc,
                tc.tile_pool(name="ffh2", bufs=2) as ffh2,
                tc.tile_pool(name="ffg", bufs=1) as ffg,
                tc.tile_pool(name="ffk", bufs=2) as ffk,
                tc.tile_pool(name="ffs", bufs=2) as ffs,
                tc.tile_pool(name="ffhf", bufs=1) as ffhf,
                tc.tile_pool(name="ffo", bufs=3) as ffo,
            ):
                ones_fb = ffc.tile([128, 1], F32, tag="ones_fb")
                nc.vector.memset(ones_fb[:], 1.0)
                ones_sb = ffc.tile([128, 1], F32R, tag="ones")
                nc.vector.tensor_copy(out=ones_sb[:], in_=ones_fb[:])
                eps1 = ffc.tile([1, 1], F32, tag="eps1")
                nc.vector.memset(eps1[:], EPS)
                w1h = ffc.tile([128, KT_E, Fc], BF16, tag="w1h")
                w3h = ffc.tile([128, KT_E, Fc], BF16, tag="w3h")
                for kt in range(KT_E):
                    nc.sync.dma_start(out=w1h[:, kt, :],
                                      in_=w1T[kt * 128:(kt + 1) * 128, :])
                    nc.sync.dma_start(out=w3h[:, kt, :],
                                      in_=w3T[kt * 128:(kt + 1) * 128, :])
                for tch in range(NCH):
                    t0 = tch * TCH
                    h2a = ffh2.tile([128, KT_E, TCH], F32, tag="h2a")
                    for kt in range(KT_E):
                        nc.sync.dma_start(
                            out=h2a[:, kt, :],
                            in_=h2_shl[tch][kt * 128:(kt + 1) * 128, :])
                    ps = gps.tile([1, TCH], F32, tag="n1")
                    for kt in range(KT_E):
                        sqc = ffk.tile([128, TCH], F32R, tag="sqc")
                        nc.scalar.activation(out=sqc[:], in_=h2a[:, kt, :],
                                             func=AF.Square)
                        nc.tensor.matmul(ps[:], ones_sb[:], sqc[:],
                                         start=(kt == 0), stop=(kt == KT_E - 1))
                    st = ffs.tile([1, TCH], F32, tag="st")
                    nc.scalar.activation(out=st[:], in_=ps[:], func=AF.Sqrt,
                                         scale=1.0 / E, bias=eps1[:])
                    r0 = ffs.tile([1, TCH], F32, tag="r0")
                    nc.vector.reciprocal(out=r0[:], in_=st[:])
                    t1 = ffs.tile([1, TCH], F32, tag="t1")
                    nc.vector.tensor_tensor(out=t1[:], in0=st[:], in1=r0[:], op=OP.mult)
                    nc.vector.tensor_scalar(out=t1[:], in0=t1[:], scalar1=-1.0,
                                            scalar2=2.0, op0=OP.mult, op1=OP.add)
                    rr = ffs.tile([1, TCH], F32, tag="rr")
                    nc.vector.tensor_tensor(out=rr[:], in0=r0[:], in1=t1[:], op=OP.mult)
                    s2b = ffs.tile([128, TCH], F32, tag="s2b")
                    nc.gpsimd.partition_broadcast(s2b[:], rr[:])
                    gc = ffg.tile([128, KT_E, TCH], BF16, tag="gc")
                    for kt in range(KT_E):
                        nc.vector.tensor_tensor(out=gc[:, kt, :], in0=h2a[:, kt, :],
                                                in1=s2b[:], op=OP.mult)
                    hff = ffhf.tile([128, FM, TCH], BF16, tag="hff")
                    for fm in range(FM):
                        ps1 = gps.tile([128, TCH], F32, tag="sc")
                        for kt in range(KT_E):
                            nc.tensor.matmul(
                                ps1[:], w1h[:, kt, fm * 128:(fm + 1) * 128],
                                gc[:, kt, :],
                                start=(kt == 0), stop=(kt == KT_E - 1))
                        h1 = ffhf.tile([128, TCH], F32, tag="h1")
                        nc.scalar.activation(out=h1[:], in_=ps1[:], func=AF.Silu)
                        ps3 = gps.tile([128, TCH], F32, tag="attv")
                        for kt in range(KT_E):
                            nc.tensor.matmul(
                                ps3[:], w3h[:, kt, fm * 128:(fm + 1) * 128],
                                gc[:, kt, :],
                                start=(kt == 0), stop=(kt == KT_E - 1))
                        nc.vector.tensor_tensor(out=hff[:, fm, :], in0=h1[:],
                                                in1=ps3[:], op=OP.mult)
                    for em in range(KT_E):
                        w2_em = ffk.tile([128, FM, 128], BF16, tag="w2_em")
                        nc.sync.dma_start(
                            out=w2_em[:],
                            in_=w2T[:].rearrange("(kf p) c -> p kf c", p=128)[
                                :, :, em * 128:(em + 1) * 128])
                        psd = gps.tile([128, TCH], F32, tag="mm")
                        for kf in range(FM):
                            nc.tensor.matmul(
                                psd[:], w2_em[:, kf, :],
                                hff[:, kf, :],
                                start=(kf == 0), stop=(kf == FM - 1))
                        od = ffo.tile([128, TCH], F32, tag="od")
                        nc.vector.scalar_tensor_tensor(
                            out=od[:], in0=h2a[:, em, :], scalar=1.0 / N_CORES,
                            in1=psd[:], op0=OP.mult, op1=OP.add)
                        nc.sync.dma_start(
                            out=outT[em * 128:(em + 1) * 128, t0:t0 + TCH],
                            in_=od[:])
            gps_cm.__exit__(None, None, None)

    if not nc.is_finalized():
        nc.finalize()
    return nc


# ---------------------------------------------------------------------------
# host side
# ---------------------------------------------------------------------------

_DEINT = np.r_[np.arange(0, 64, 2), np.arange(1, 64, 2)]


def _prep_inputs(x, freqs_cis, w_qkv, w_o, w1, w2, w3, attn_norm_w, ff_norm_w, cfg):
    c = cfg
    B, S, E, F, T = c["B"], c["S"], c["E"], c["F"], c["T"]
    H, KH, HD = 32, 8, 64
    KV = KH * HD

    x2 = np.asarray(x, dtype=np.float32).reshape(T, E)
    xT = np.ascontiguousarray(x2.T)
    xTb = xT.astype(ml_dtypes.bfloat16)

    fc = np.asarray(freqs_cis, dtype=np.float32)       # [S, 32, 2]
    cos32 = np.ascontiguousarray(fc[:, :, 0].T)        # [32, S]
    sin32 = np.ascontiguousarray(fc[:, :, 1].T)
    cosb = np.concatenate([cos32] * B, axis=1)         # [32, T]
    sinb = np.concatenate([sin32] * B, axis=1)
    cosq = np.tile(cosb, (4, 1))                       # [128, T]
    sinq = np.concatenate([-sinb, sinb, -sinb, sinb], axis=0)

    n1 = np.asarray(attn_norm_w, dtype=np.float32)
    n2 = np.asarray(ff_norm_w, dtype=np.float32)
    wq = np.asarray(w_qkv[:E], dtype=np.float32).reshape(H, HD, E)
    wk = np.asarray(w_qkv[E:E + KV], dtype=np.float32).reshape(KH, HD, E)
    wv = np.asarray(w_qkv[E + KV:], dtype=np.float32).reshape(KH, HD, E)
    w_o = np.asarray(w_o, dtype=np.float32)
    w1 = np.asarray(w1, dtype=np.float32)
    w3 = np.asarray(w3, dtype=np.float32)
    w2 = np.asarray(w2, dtype=np.float32)

    in_maps = []
    Fc = F // N_CORES
    for core in range(N_CORES):
        rows = []
        for j in range(4):
            rows.append(wq[core * 4 + j][_DEINT] * 0.125)
        kd = wk[core][_DEINT]
        rows += [kd, kd, wv[core]]
        wsh = np.concatenate(rows, axis=0) * n1[None, :]        # [448, E]
        wqkvT_np = np.ascontiguousarray(wsh.T).astype(ml_dtypes.bfloat16)
        woT_np = np.ascontiguousarray(w_o[:, core * 256:(core + 1) * 256].T)
        fsl = slice(core * Fc, (core + 1) * Fc)
        w1T_np = np.ascontiguousarray((w1[fsl] * n2[None, :]).T).astype(ml_dtypes.bfloat16)
        w3T_np = np.ascontiguousarray((w3[fsl] * n2[None, :]).T).astype(ml_dtypes.bfloat16)
        w2T_np = np.ascontiguousarray(w2[:, fsl].T).astype(ml_dtypes.bfloat16)
        in_maps.append({
            "xT": xT, "xTb": xTb, "wqkvT": wqkvT_np, "woT": woT_np,
            "w1T": w1T_np, "w3T": w3T_np, "w2T": w2T_np,
            "cosq": cosq, "sinq": sinq,
        })
    return in_maps


_BUILD_CACHE = {}


def _get_nc(cfg_key):
    if cfg_key not in _BUILD_CACHE:
        _BUILD_CACHE[cfg_key] = build(_cfg(*cfg_key))
    return _BUILD_CACHE[cfg_key]


def run(x, freqs_cis, w_qkv, w_o, w1, w2, w3, attn_norm_w, ff_norm_w,
        S=2048, F=8192):
    from concourse.bass_utils import run_bass_kernel_spmd
    cfg = _cfg(S, F)
    in_maps = _prep_inputs(x, freqs_cis, w_qkv, w_o, w1, w2, w3,
                           attn_norm_w, ff_norm_w, cfg)
    nc = _get_nc((S, F))
    res = run_bass_kernel_spmd(nc, in_maps, core_ids=list(range(N_CORES)))
    acc = np.zeros((cfg["E"], cfg["T"]), dtype=np.float64)
    for r in res.results:
        acc += r["outT"].astype(np.float64)
    out = acc.T.reshape(cfg["B"], S, cfg["E"]).astype(np.float32)
    return out


def kernel(x, attention_mask, freqs_cis, w_qkv, w_o, w1, w2, w3,
           attn_norm_w, ff_norm_w):
    return run(x, freqs_cis, w_qkv, w_o, w1, w2, w3, attn_norm_w, ff_norm_w,
               S=2048, F=8192)

